# revision 1
# baseline (speedup 1.0000x reference)
"""Multi-head attention (B=2, S=2048, D=1024, H=16, Dk=64) on 8 NeuronCores.

Sharding: 2-way data parallel over batch x 4-way tensor parallel over heads.
Core c handles batch c//4 and heads (c%4)*4 .. (c%4)*4+3. Each core computes
a partial output projection [S, D] in bf16; the host sums the 4 partials per
batch in fp32 and adds bo.

v4 over v3 (trace-driven):
- normalize is PE+DVE only. The SWDGE scatter DMAs ([1,512]<->[128,4]) used
  for the denominator transpose measured ~10us completion each and stalled
  the PE FIFO ~10.5us at every block boundary. Now: K=1 outer-product
  matmuls lift the denominator row into partition-major PSUM, a 128-lane
  reciprocal runs on DVE, a PE transpose + zero-padded selector matmuls
  broadcast the reciprocals back across partitions. All prompt, so the
  whole chain is emitted inline at the block end.
- Biases and small constants load before the big weights (they gated the
  first exp at ~32us); wk/wq ride the otherwise-idle ACT HWDGE ring in
  parallel with x^T on the SP ring.
- Dependency-free warm-up matmuls on zeroed tiles hold the PE clock at
  2.4GHz through the initial DMA wait.
"""
from contextlib import ExitStack

import numpy as np
import ml_dtypes
import concourse.bass as bass
import concourse.mybir as mybir
import concourse.tile as tile
from concourse import bacc
from concourse.bass_utils import run_bass_kernel_spmd

f32 = mybir.dt.float32
bf16 = mybir.dt.bfloat16
AF = mybir.ActivationFunctionType

B, S, D = 2, 2048, 1024
H, DK = 16, 64
NCORES = 8
TP = 4                 # tensor-parallel factor (head groups)
HPC = H // TP          # 4 heads per core
DP = HPC * DK          # 256 = per-core d' slice
SBK = 512              # s-block for attention streaming
NJ = S // SBK          # 4
NT = S // 128          # 16 t-tiles
NDC = D // 128         # 8 contraction chunks over D
NPC = DP // 128        # 2 chunks over d'

_prog_cache = {}


def _build_program():
    nc = bacc.Bacc()
    xt = nc.dram_tensor("xt", [128, NJ, NDC, SBK], bf16, kind="ExternalInput")
    wq = nc.dram_tensor("wq", [128, NDC, DP], bf16, kind="ExternalInput")
    wk = nc.dram_tensor("wk", [128, NDC, DP], bf16, kind="ExternalInput")
    wv = nc.dram_tensor("wv", [128, NDC, DP], bf16, kind="ExternalInput")
    wo = nc.dram_tensor("wo", [128, NPC, D], bf16, kind="ExternalInput")
    bq = nc.dram_tensor("bq", [128, NPC], f32, kind="ExternalInput")
    bk = nc.dram_tensor("bk", [128, NPC], f32, kind="ExternalInput")
    bv = nc.dram_tensor("bv", [128, DP], f32, kind="ExternalInput")
    ident = nc.dram_tensor("ident", [128, 128], bf16, kind="ExternalInput")
    # sel8[c', k, p] = 1 iff c' == 2k + (p >= 64), rows 8-127 zero
    sel8 = nc.dram_tensor("sel8", [128, 4, 128], bf16, kind="ExternalInput")
    out = nc.dram_tensor("out", [S, D], bf16, kind="ExternalOutput")

    with tile.TileContext(nc) as tc, ExitStack() as top:
        const = top.enter_context(tc.tile_pool(name="const", bufs=1))
        big = top.enter_context(tc.tile_pool(name="big", bufs=1))

        # persistent weights / biases
        wq_r = const.tile([128, NDC, DP], bf16)
        wk_r = const.tile([128, NDC, DP], bf16)
        wv_r = const.tile([128, NDC, DP], bf16)
        wo_r = const.tile([128, NPC, D], bf16)
        bq_sb = const.tile([128, NPC], f32)
        bk_sb = const.tile([128, NPC], f32)
        bv_b = const.tile([128, DP], f32)
        id_sb = const.tile([128, 128], bf16)
        sel_sb = const.tile([128, 4, 128], bf16)
        one1 = const.tile([1, 1], bf16)
        wup = const.tile([128, 128], bf16)
        rsb = const.tile([128, 128], bf16)
        zro = const.tile([128, SBK], bf16)
        dum_i = const.tile([128, 8], f32)
        dum_o = const.tile([128, 8], f32)

        # persistent activations
        xt_r = big.tile([128, NJ, NDC, SBK], bf16)
        qt = big.tile([128, NPC, S], bf16)
        kt = big.tile([128, NPC, S], bf16)
        vaug = big.tile([128, NT, HPC, DK + 1], bf16)
        ct = big.tile([128, NPC, S], bf16)

        esp = top.enter_context(tc.tile_pool(name="esp", bufs=4))
        smal = top.enter_context(tc.tile_pool(name="smal", bufs=2))
        outp = top.enter_context(tc.tile_pool(name="outp", bufs=2))
        ps_s = top.enter_context(tc.tile_pool(name="ps_s", bufs=2, space="PSUM"))
        ps_c = top.enter_context(tc.tile_pool(name="ps_c", bufs=1, space="PSUM"))
        ps_x = top.enter_context(tc.tile_pool(name="ps_x", bufs=2, space="PSUM"))

        # preload the exp table set while DMAs run
        nc.vector.memset(dum_i, 0.0)
        nc.scalar.activation(out=dum_o, in_=dum_i, func=AF.Exp)
        nc.vector.memset(one1, 1.0)
        nc.vector.memset(wup, 0.0)
        nc.vector.memset(zro, 0.0)
        nc.vector.memset(rsb, 0.0)

        # ---- loads: biases/consts first (tiny), then weights; wk/wq on the
        # ACT HWDGE ring so they land in parallel with x^T on the SP ring
        nc.gpsimd.dma_start(out=bq_sb, in_=bq[:])
        nc.gpsimd.dma_start(out=bk_sb, in_=bk[:])
        nc.gpsimd.dma_start(out=bv_b, in_=bv[:])
        nc.gpsimd.dma_start(out=id_sb, in_=ident[:])
        nc.gpsimd.dma_start(out=sel_sb, in_=sel8[:])
        nc.sync.dma_start(out=wk_r, in_=wk[:])
        nc.sync.dma_start(out=wq_r, in_=wq[:])
        for j in range(NJ):
            nc.sync.dma_start(out=xt_r[:, j], in_=xt[:, j])
        nc.gpsimd.dma_start(out=wv_r, in_=wv[:])
        nc.gpsimd.dma_start(out=wo_r, in_=wo[:])

        # ones column of V-augmented (denominator trick)
        for t in range(NT):
            nc.vector.memset(vaug[:, t, :, DK], 1.0)

        # ---- HAM pre-warm: dependency-free matmuls on zeroed tiles keep
        # the PE busy through the initial DMA wait so the clock is at 2.4GHz
        # when the first projections land
        for w in range(2):
            warm = ps_x.tile([128, SBK], f32, tag="px", name=f"warm{w}")
            for r in range(8):
                nc.tensor.matmul(out=warm[:, 0:128], lhsT=wup, rhs=zro[:, 0:128],
                                 start=(r == 0), stop=(r == 7))

        def proj_qk(wr, bias_sb, dst, c, j):
            pq = ps_x.tile([128, SBK], f32, tag="px", name=f"pj{id(wr) % 97}_{c}_{j}")
            for k in range(NDC):
                nc.tensor.matmul(
                    out=pq,
                    lhsT=wr[:, k, c * 128:(c + 1) * 128],
                    rhs=xt_r[:, j, k, :],
                    start=(k == 0), stop=(k == NDC - 1),
                )
            nc.vector.tensor_scalar_add(
                out=dst[:, c, j * SBK:(j + 1) * SBK],
                in0=pq, scalar1=bias_sb[:, c:c + 1],
            )

        def proj_v(t):
            pv = ps_x.tile([128, DP], f32, tag="px", name=f"pv{t}")
            for k in range(NDC):
                nc.tensor.matmul(
                    out=pv,
                    lhsT=xt_r[:, t // 4, k, (t % 4) * 128:(t % 4 + 1) * 128],
                    rhs=wv_r[:, k, :],
                    start=(k == 0), stop=(k == NDC - 1),
                )
            nc.vector.tensor_add(
                out=vaug[:, t, :, 0:DK],
                in0=pv.rearrange("p (h d) -> p h d", h=HPC),
                in1=bv_b.rearrange("p (h d) -> p h d", h=HPC),
            )

        # ---- lead-in: K/Q projections for (c=0, j=0) gate the first scores
        proj_qk(wk_r, bk_sb, kt, 0, 0)
        proj_qk(wq_r, bq_sb, qt, 0, 0)

        # ---- attention + output projection ----
        from collections import defaultdict
        filler = []
        sched = defaultdict(list)
        now = {"g": 0}

        def feed(n):
            for _ in range(min(n, len(filler))):
                filler.pop(0)()

        def flush():
            while filler:
                filler.pop(0)()

        def queue_proj_qk(wr, bias_sb, dst, c, j):
            state = {}
            def mk(k):
                def go():
                    if k == 0:
                        state["pq"] = ps_x.tile(
                            [128, SBK], f32, tag="px", name=f"fq{c}_{j}_{id(wr) % 97}")
                    nc.tensor.matmul(
                        out=state["pq"],
                        lhsT=wr[:, k, c * 128:(c + 1) * 128],
                        rhs=xt_r[:, j, k, :],
                        start=(k == 0), stop=(k == NDC - 1),
                    )
                    if k == NDC - 1:
                        nc.vector.tensor_scalar_add(
                            out=dst[:, c, j * SBK:(j + 1) * SBK],
                            in0=state["pq"], scalar1=bias_sb[:, c:c + 1],
                        )
                return go
            for k in range(NDC):
                filler.append(mk(k))

        def queue_outproj(j, scalar_evac=False):
            for stj in range(SBK // 128):
                st = j * (SBK // 128) + stj
                state = {}
                def mk(c, nh, st=st, state=state):
                    def go():
                        if c == 0 and nh == 0:
                            state["ob"] = outp.tile(
                                [128, D], bf16, tag="ob", name=f"ob{st}")
                        if c == 0:
                            state["po"] = ps_x.tile(
                                [128, 512], f32, tag="px", name=f"po{st}_{nh}")
                        nc.tensor.matmul(
                            out=state["po"],
                            lhsT=ct[:, c, st * 128:(st + 1) * 128],
                            rhs=wo_r[:, c, nh * 512:(nh + 1) * 512],
                            start=(c == 0), stop=(c == NPC - 1),
                        )
                        if c == NPC - 1:
                            if scalar_evac and nh == 0:
                                nc.scalar.copy(
                                    out=state["ob"][:, nh * 512:(nh + 1) * 512],
                                    in_=state["po"])
                            else:
                                nc.vector.tensor_copy(
                                    out=state["ob"][:, nh * 512:(nh + 1) * 512],
                                    in_=state["po"])
                            if nh == 1:
                                nc.sync.dma_start(
                                    out=out[st * 128:(st + 1) * 128, :],
                                    in_=state["ob"])
                    return go
                for nh in range(2):
                    for c in range(NPC):
                        filler.append(mk(c, nh, st=st, state=state))

        def normalize(j, hp, pcs):
            # staged across the next 5 steps so the PE/DVE ping-pong never
            # lumps between two score matmuls and starves the exp stream
            g = now["g"]
            cu = smal.tile([DK + 1, 2, SBK], f32, tag="cu", name=f"cu{j}{hp}")
            dn0 = smal.tile([1, 2, SBK], bf16, tag="dn0", name=f"dn0{j}{hp}")
            for hh in range(2):
                nc.vector.tensor_copy(out=cu[:, hh, :], in_=pcs[hh])
                nc.vector.tensor_copy(out=dn0[:, hh, :], in_=pcs[hh][DK:DK + 1, :])
            dnP_t = {}
            rT = smal.tile([128, 8], bf16, tag="rT", name=f"rT{j}{hp}")
            rb_t = {}

            def st1():
                dnP = ps_x.tile([128, 8], f32, tag="px", name=f"dnP{j}{hp}")
                dnP_t["x"] = dnP
                for hh in range(2):
                    for k in range(4):
                        cc = 2 * k + hh
                        nc.tensor.matmul(
                            out=dnP[:, cc:cc + 1],
                            lhsT=dn0[0:1, hh, k * 128:(k + 1) * 128],
                            rhs=one1,
                            start=True, stop=True,
                        )

            def st2():
                with nc.allow_low_precision(reason="bf16 softmax denominators"):
                    nc.vector.reciprocal(out=rT, in_=dnP_t["x"])
                rTT = ps_x.tile([8, 128], bf16, tag="px", name=f"rTT{j}{hp}")
                nc.tensor.transpose(out=rTT, in_=rT, identity=id_sb)
                nc.vector.tensor_copy(out=rsb[0:8, :], in_=rTT)

            def st3():
                rb = ps_x.tile([128, SBK], f32, tag="px", name=f"rb{j}{hp}")
                rb_t["x"] = rb
                for k in range(4):
                    nc.tensor.matmul(
                        out=rb[:, k * 128:(k + 1) * 128],
                        lhsT=sel_sb[:, k, :], rhs=rsb,
                        start=True, stop=True,
                    )

            def st4():
                for hh in range(2):
                    nc.vector.tensor_mul(
                        out=ct[hh * 64:(hh + 1) * 64, hp, j * SBK:(j + 1) * SBK],
                        in0=cu[0:DK, hh, :],
                        in1=rb_t["x"][hh * 64:(hh + 1) * 64, :],
                    )

            sched[g + 1].append(st1)
            sched[g + 2].append(st2)
            sched[g + 3].append(st3)
            sched[g + 4].append(st4)
            if hp == 1:
                sched[g + 5].append(
                    lambda j=j: queue_outproj(j, scalar_evac=(j == NJ - 1)))

        # Software-pipelined driver (depth 3): at step i emit S(i), exp(i),
        # then PV of step i-3.
        pend = []

        def drain_pv():
            if not pend:
                return
            j, hp, t, es, pcs = pend.pop(0)
            for hh in range(2):
                nc.tensor.matmul(
                    out=pcs[hh],
                    lhsT=vaug[:, t, hp * 2 + hh, :],
                    rhs=es[:, hh, :],
                    start=(t == 0), stop=(t == NT - 1),
                )
            if t == NT - 1:
                normalize(j, hp, pcs)

        pcs_by = {}
        for j in range(NJ):
            for hp in range(NPC):
                pcs_by[(j, hp)] = [
                    ps_c.tile([DK + 1, SBK], f32, tag=f"pc{hh}", name=f"pc{hh}_{j}_{hp}")
                    for hh in range(2)]
                if (j, hp) == (0, 0):
                    # K(c0,jj) by step 4jj, then K(c1,0)+Q(c1,0) by step 16
                    for jj in range(1, NJ):
                        queue_proj_qk(wk_r, bk_sb, kt, 0, jj)
                    queue_proj_qk(wk_r, bk_sb, kt, 1, 0)
                    queue_proj_qk(wq_r, bq_sb, qt, 1, 0)
                if (j, hp) == (0, 1):
                    # K(c1,jj) by step 16+4jj
                    for jj in range(1, NJ):
                        queue_proj_qk(wk_r, bk_sb, kt, 1, jj)
                if hp == 1 and j + 1 < NJ:
                    for c in range(NPC):
                        queue_proj_qk(wq_r, bq_sb, qt, c, j + 1)
                for t in range(NT):
                    ss = ps_s.tile([128, 2, SBK], f32, tag="ss", name=f"ss{j}_{hp}_{t}")
                    for hh in range(2):
                        nc.tensor.matmul(
                            out=ss[:, hh, :],
                            lhsT=kt[hh * 64:(hh + 1) * 64, hp, t * 128:(t + 1) * 128],
                            rhs=qt[hh * 64:(hh + 1) * 64, hp, j * SBK:(j + 1) * SBK],
                            start=True, stop=True,
                        )
                    es = esp.tile([128, 2, SBK], bf16, tag="es", name=f"es{j}_{hp}_{t}")
                    nc.scalar.activation(out=es, in_=ss, func=AF.Exp, scale=0.125)
                    for fn in sched.pop(now["g"], []):
                        fn()
                    if j == 0 and hp == 0:
                        proj_v(t)             # V projection rides along
                    depth = 1 if (j, hp) == (NJ - 1, NPC - 1) and t >= 13 else 3
                    if len(pend) >= depth:
                        drain_pv()
                    pend.append((j, hp, t, es, pcs_by[(j, hp)]))
                    if j == 0 and hp == 0:
                        feed(2 if t < 12 else 4)
                    else:
                        feed(2)
                    now["g"] += 1
        while pend:
            drain_pv()
        for g in sorted(sched):
            for fn in sched[g]:
                fn()
        sched.clear()
        flush()

    nc.finalize()
    return nc


def _get_program():
    if "nc" not in _prog_cache:
        _prog_cache["nc"] = _build_program()
    return _prog_cache["nc"]


def _make_in_maps(x, Wq, bq, Wk, bk, Wv, bv, Wo, bo):
    bfdt = ml_dtypes.bfloat16
    ident = np.eye(128, dtype=bfdt)
    sel8 = np.zeros((128, 4, 128), dtype=bfdt)
    for k in range(4):
        sel8[2 * k, k, 0:64] = 1.0
        sel8[2 * k + 1, k, 64:128] = 1.0
    in_maps = []
    xt_by_b = []
    for b in range(B):
        # xt[p, j, k, s'] = x[b][j*SBK+s', k*128+p]
        xtb = np.ascontiguousarray(
            x[b].reshape(NJ, SBK, NDC, 128).transpose(3, 0, 2, 1).astype(bfdt))
        xt_by_b.append(xtb)
    for c in range(NCORES):
        b, hg = divmod(c, TP)
        sl = slice(hg * DP, (hg + 1) * DP)
        in_maps.append({
            "xt": xt_by_b[b],
            # w[p, k, d'] = W[k*128+p, d']
            "wq": np.ascontiguousarray(
                Wq[:, sl].reshape(NDC, 128, DP).transpose(1, 0, 2).astype(bfdt)),
            "wk": np.ascontiguousarray(
                Wk[:, sl].reshape(NDC, 128, DP).transpose(1, 0, 2).astype(bfdt)),
            "wv": np.ascontiguousarray(
                Wv[:, sl].reshape(NDC, 128, DP).transpose(1, 0, 2).astype(bfdt)),
            # wo[p, c, dout] = Wo[sl][c*128+p, dout]
            "wo": np.ascontiguousarray(
                Wo[sl, :].reshape(NPC, 128, D).transpose(1, 0, 2).astype(bfdt)),
            "bq": np.ascontiguousarray(bq[sl].reshape(NPC, 128).T),
            "bk": np.ascontiguousarray(bk[sl].reshape(NPC, 128).T),
            "bv": np.ascontiguousarray(
                np.broadcast_to(bv[sl][None, :], (128, DP)).copy()),
            "ident": ident,
            "sel8": sel8,
        })
    return in_maps


def run(inputs, **spmd_kwargs):
    """Build, run on 8 cores, gather. Returns (output, BassKernelResults)."""
    args = {k: np.asarray(v, dtype=np.float32) for k, v in inputs.items()}
    nc = _get_program()
    in_maps = _make_in_maps(
        args["x"], args["Wq"], args["bq"], args["Wk"], args["bk"],
        args["Wv"], args["bv"], args["Wo"], args["bo"],
    )
    res = run_bass_kernel_spmd(nc, in_maps, list(range(NCORES)), **spmd_kwargs)
    out = np.zeros((B, S, D), dtype=np.float32)
    for c in range(NCORES):
        b = c // TP
        out[b] += np.asarray(res.results[c]["out"]).astype(np.float32)
    out += args["bo"]
    return out, res


def kernel(**inputs):
    out, _ = run(inputs)
    return out

